# revision 45
# baseline (speedup 1.0000x reference)
"""Bass/Trainium2 kernel for nn_Head_13030930776875 (v3: interleaved).

out = 0.7*softmax(causal(q k^T / sqrt(d))) @ v
    + 0.3*rownorm(causal(exp(-|y_i-y_j|^2 / (2d)))) @ v,   y = k @ L_grav

Sharding: 8 cores = 4 samples x 2 halves. Half h owns query blocks
{h, h+2, ..., h+14} (128-row blocks, stride-2 interleave) — rank-matched
causal needs, so the uniform trim covers both halves with only 72 of the
136 possible 128x128 score tiles per attention (optimal for 2 cores).

Per-core key order is PERMUTED host-side (group g packed as
[4g+h, 4g+1-h, 4g+2+h, 4g+3-h]) so the owned query blocks sit at even
packed positions — queries are strided views of x^T / y^T, no separate
query tensors. Attention over keys is order-invariant; causal masks are
per-core data (one [128,128] mask slot per chunk covers the single
band-or-invalid leftmost sub-block).

Scales are folded host-side (Wq/sqrt(d), L/sqrt(d)) so lang+grav exp
merge into ONE bias-free ACT call per chunk over adjacent PSUM banks;
the grav per-key factor exp(-sq_k/2) rides a g-scaled copy of v_aug.
"""

import math
import os

import numpy as np

B, N, D_MODEL, D_HEAD = 4, 2048, 1024, 128
OMEGA_LANG, OMEGA_GRAV = 0.7, 0.3
NBLK = 16

_CACHE = {}


def _build_nc():
    import concourse.bacc as bacc
    import concourse.mybir as mybir
    import concourse.tile as tile

    dt = mybir.dt
    F16, F32 = dt.float16, dt.float32
    AF = mybir.ActivationFunctionType
    OP = mybir.AluOpType

    nc = bacc.Bacc()

    sm16 = nc.declare_dram_parameter("sm16", [128, 272], F16, isOutput=False)
    wkd = nc.declare_dram_parameter("wkd", [128, 1024], F16, isOutput=False)
    wqv = nc.declare_dram_parameter("wqv", [128, 2 * 1024], F16, isOutput=False)
    # xg[g*128+p, ((c*4+s)*128)+n] = xT[c*128+p, P_g[s]*128+n], permuted blocks
    xg = nc.declare_dram_parameter("xg", [4 * 128, 8 * 512], F16, isOutput=False)
    out_d = nc.declare_dram_parameter("out", [N // 2, 128], F32, isOutput=True)

    with tile.TileContext(nc) as tc:
        with (
            tc.tile_pool(name="big", bufs=1) as big,
            tc.tile_pool(name="xtp", bufs=1) as xtp,
            tc.tile_pool(name="ap", bufs=2) as apool,
            tc.tile_pool(name="small", bufs=4) as small,
            tc.tile_pool(name="outp", bufs=2) as outp,
            tc.tile_pool(name="score", bufs=2, space="PSUM") as score,
            tc.tile_pool(name="pp", bufs=4, space="PSUM") as pp,
        ):
            # PE warmup first: matmuls on a memset tile (no DMA dependency)
            # keep the HAM clock-gate open until real work arrives.
            wtile = big.tile([128, 128], F16, tag="wt")
            nc.vector.memset(wtile[:], 0.5)
            warm = pp.tile([128, 512], F32, tag="pp")
            for i in range(64):
                nc.tensor.matmul(warm[:, 0:128], wtile[:], wtile[:],
                                 start=(i == 0), stop=(i == 63))

            # ---- xg0 first (it gates the first kT proj), then wk/smalls ----
            # x^T permuted-block layout (g, c, j, o, n): o=0 are owned q blocks
            xt_all = xtp.tile([128, 4, 8, 2, 2, 128], F16, tag="xt")

            def load_xg(g):
                nc.sync.dma_start(
                    xt_all[:, g],
                    xg[g * 128:(g + 1) * 128, :].rearrange(
                        "p (c j o n) -> p c j o n", c=8, j=2, o=2))

            load_xg(0)
            wk_s = big.tile([128, 8, 128], F16, tag="wk")
            nc.sync.dma_start(wk_s[:], wkd[:].rearrange("p (c d) -> p c d", c=8))
            sm16_s = big.tile([128, 272], F16, tag="sm16")
            nc.sync.dma_start(sm16_s[:], sm16[:])
            lg_s = sm16_s[:, 0:128]
            io_s = sm16_s[:, 128:256]
            th_s = big.tile([128, 16], F32, tag="th")
            nc.vector.tensor_copy(th_s[:], sm16_s[:, 256:272])
            load_xg(1)
            wp_s = big.tile([128, 2, 8, 128], F16, tag="wp")
            nc.sync.dma_start(wp_s[:], wqv[:].rearrange("p (w c d) -> p w c d",
                                                        w=2, c=8))
            load_xg(2)
            load_xg(3)

            kT = big.tile([128, 16, 128], F16, tag="kT")
            yT = big.tile([128, 8, 2, 128], F16, tag="yT")
            qT = big.tile([128, 1024], F16, tag="qT")
            sqn = big.tile([128, NBLK], F32, tag="sqn")
            g_s = big.tile([128, NBLK], F32, tag="gs")
            vaug = big.tile([128, NBLK, 132], F16, tag="vaug")
            vaug_g = big.tile([128, NBLK, 132], F16, tag="vaugg")

            # causal masks: mk[slot] = (iota128 >= thr[slot]), 16 slots
            mk_s = big.tile([128, 16, 128], F16, tag="mk")
            for slot in range(16):
                nc.vector.tensor_scalar(mk_s[:, slot, :], io_s,
                                        th_s[:, slot:slot + 1], None, OP.is_ge)

            def proj_kt(g):
                ps = pp.tile([128, 512], F32, tag="pp")
                for c in range(8):
                    nc.tensor.matmul(ps[:], wk_s[:, c, :], xt_all[:, g, c],
                                     start=(c == 0), stop=(c == 7))
                nc.vector.tensor_copy(kT[:, 4 * g:4 * (g + 1), :], ps[:])

            def proj_qt(half):
                ps = pp.tile([128, 512], F32, tag="pp")
                for c in range(8):
                    nc.tensor.matmul(ps[:],
                                     wp_s[:, 0, c, :],
                                     xt_all[:, 2 * half:2 * half + 2, c, :, 0, :],
                                     start=(c == 0), stop=(c == 7))
                nc.vector.tensor_copy(qT[:, half * 512:(half + 1) * 512], ps[:])

            def yt_group(g):
                ps = pp.tile([128, 512], F32, tag="pp")
                nc.tensor.matmul(ps[:], lg_s, kT[:, 4 * g:4 * (g + 1), :])
                nc.vector.tensor_copy(yT[:, 2 * g:2 * (g + 1), :, :], ps[:])

            def sqn_chunk(kb):
                ps = pp.tile([128, 512], F32, tag="pp")
                nc.tensor.matmul(ps[:, 0:128], kT[:, kb, :], lg_s)
                scr = small.tile([128, 128], F32, tag="scr")
                nc.scalar.activation(scr[:], ps[:, 0:128], AF.Square,
                                     scale=0.70710678, accum_out=sqn[:, kb:kb + 1])

            def vaug_chunk(kb):
                g, s = kb // 4, kb % 4
                ps = pp.tile([128, 512], F32, tag="pp")
                for c in range(8):
                    nc.tensor.matmul(ps[:, 0:128],
                                     xt_all[:, g, c, s // 2, s % 2, :],
                                     wp_s[:, 1, c, :], start=(c == 0), stop=(c == 7))
                nc.vector.tensor_copy(vaug[:, kb, 0:128], ps[:, 0:128])
                nc.vector.memset(vaug[:, kb, 128:129], 1.0)
                nc.vector.tensor_scalar(vaug_g[:, kb, 0:129], vaug[:, kb, 0:129],
                                        g_s[:, kb:kb + 1], None, OP.mult)

            def chunk(pos, kb, amrg):
                shrink = ((kb // 2) * 128 if pos == 0
                          else max(0, kb // 2 - 4) * 128)
                qsl = qT[:, pos * 512 + shrink:pos * 512 + 512]
                i0 = pos * 4 + shrink // 128
                ysl = yT[:, i0:pos * 4 + 4, 0, :]
                sc = score.tile([128, 1024], F32, tag="sc")
                nc.tensor.matmul(sc[:, shrink:512], kT[:, kb, :], qsl)
                nc.tensor.matmul(sc[:, 512 + shrink:1024],
                                 yT[:, kb // 2, kb % 2, :], ysl)
                nc.scalar.activation(amrg[:, kb, shrink:1024],
                                     sc[:, shrink:1024], AF.Exp)
                # leftmost included sub-block is band/invalid/full per-core
                if pos == 0 or kb >= 8:
                    nc.vector.tensor_tensor(
                        amrg[:, kb, shrink:shrink + 128],
                        amrg[:, kb, shrink:shrink + 128],
                        mk_s[:, kb, :], OP.mult)
                    nc.vector.tensor_tensor(
                        amrg[:, kb, 512 + shrink:512 + shrink + 128],
                        amrg[:, kb, 512 + shrink:512 + shrink + 128],
                        mk_s[:, kb, :], OP.mult)

            def av_block(pos, jj, amrg, obp):
                nkb = 2 * jj + 2 if pos == 0 else 2 * jj + 10
                pol = pp.tile([128, 132], F32, tag="pp")
                pog = pp.tile([128, 132], F32, tag="pp")
                for kb in range(nkb):
                    nc.tensor.matmul(pol[:, 0:129],
                                     amrg[:, kb, jj * 128:(jj + 1) * 128],
                                     vaug[:, kb, 0:129],
                                     start=(kb == 0), stop=(kb == nkb - 1))
                for kb in range(nkb):
                    nc.tensor.matmul(pog[:, 0:129],
                                     amrg[:, kb, 512 + jj * 128:512 + (jj + 1) * 128],
                                     vaug_g[:, kb, 0:129],
                                     start=(kb == 0), stop=(kb == nkb - 1))
                rl = small.tile([128, 1], F32, tag="rl")
                rg = small.tile([128, 1], F32, tag="rg")
                nc.vector.reciprocal(rl[:], pol[:, 128:129])
                nc.vector.tensor_scalar(rl[:], rl[:], OMEGA_LANG, None, OP.mult)
                nc.vector.reciprocal(rg[:], pog[:, 128:129])
                nc.vector.tensor_scalar(rg[:], rg[:], OMEGA_GRAV, None, OP.mult)
                ob = outp.tile([128, 128], F32, tag="ob")
                nc.vector.tensor_scalar(ob[:], pol[:, 0:128], rl[:], None, OP.mult)
                nc.vector.scalar_tensor_tensor(obp[:, jj % 2, :], pog[:, 0:128],
                                               rg[:], ob[:], OP.mult, OP.add)

            def store_pair(pos, pair, obp):
                r0 = pos * 512 + pair * 256
                nc.sync.dma_start(
                    out_d[r0:r0 + 256, :].rearrange("(j p) d -> p j d", j=2),
                    obp[:])

            # ---- attention: pos0 = local query blocks 0..3, pos1 = 4..7 ----
            # AV for block jj is emitted right after its last chunk (nkb-1)
            # so the PE fills EXP gaps and outputs stream out early.
            amrg0 = apool.tile([128, 16, 1024], F16, tag="amrg", name="amrg0")
            amrg1 = apool.tile([128, 16, 1024], F16, tag="amrg", name="amrg1")

            # pos0 prerequisites
            proj_kt(0)
            proj_kt(1)
            proj_qt(0)
            yt_group(0)
            yt_group(1)
            for kb in range(8):
                sqn_chunk(kb)
            nc.scalar.activation(g_s[:, 0:8], sqn[:, 0:8], AF.Exp, scale=-1.0)

            chunk(0, 0, amrg0)
            chunk(0, 1, amrg0)
            # vaug emitted late + spread: fills PE while EXPs drain
            for kb in range(2, 8):
                vaug_chunk(kb - 2)
                chunk(0, kb, amrg0)
            vaug_chunk(6)
            vaug_chunk(7)

            # pos1 prerequisites hoisted before pos0's AVs: they gate pos1's
            # first EXP, while pos0 stores are far from the critical path.
            # sqn before proj_qt(1): qT's xg3 wait must not block the Squares.
            proj_kt(2)
            proj_kt(3)
            for kb in range(8, 16):
                sqn_chunk(kb)
            nc.scalar.activation(g_s[:, 8:16], sqn[:, 8:16], AF.Exp, scale=-1.0)
            yt_group(2)
            yt_group(3)
            proj_qt(1)

            obp = outp.tile([128, 2, 128], F32, tag="obp")
            for jj in range(4):
                av_block(0, jj, amrg0, obp)
                if jj % 2 == 1:
                    store_pair(0, jj // 2, obp)
                    obp = outp.tile([128, 2, 128], F32, tag="obp")
            chunk(1, 0, amrg1)
            chunk(1, 1, amrg1)
            for kb in range(2, 10):
                vaug_chunk(kb + 6)
                chunk(1, kb, amrg1)
            for kb in range(10, 16):
                chunk(1, kb, amrg1)
            av_block(1, 0, amrg1, obp)
            av_block(1, 1, amrg1, obp)
            store_pair(1, 0, obp)
            obp = outp.tile([128, 2, 128], F32, tag="obp")
            av_block(1, 2, amrg1, obp)
            nc.sync.dma_start(out_d[768:896, :], obp[:, 0, :])
            av_block(1, 3, amrg1, obp)
            nc.sync.dma_start(out_d[896:1024, :], obp[:, 1, :])

    nc.finalize()
    return nc


def _host_inputs(x, Wq, Wk, Wv, L_grav):
    """Build the 8 per-core input maps."""
    f16 = np.float16
    x = np.asarray(x, np.float32)
    s = 1.0 / math.sqrt(D_HEAD)
    Wq = np.asarray(Wq, np.float32) * s        # fold 1/sqrt(d) into Wq
    Wk = np.asarray(Wk, np.float32)
    Wv = np.asarray(Wv, np.float32)
    L = np.asarray(L_grav, np.float32) * s     # fold 1/sqrt(d) into L

    def warr(w):  # [1024,128] -> [128, 8*128] chunk-major for lhsT slices
        return np.ascontiguousarray(
            w.reshape(8, 128, 128).transpose(1, 0, 2).reshape(128, 8 * 128)
        ).astype(f16)

    wkd = warr(Wk)
    wqv = np.concatenate([warr(Wq), warr(Wv)], axis=1)
    iota = np.ascontiguousarray(
        np.broadcast_to(np.arange(128, dtype=np.float32), (128, 128))).astype(f16)

    def perm(h):  # packed block order within each group
        return [h, 1 - h, 2 + h, 3 - h]

    def half_thr(h):
        """thr [128, 16]: mask = iota128 >= thr, one slot per masked chunk.

        Slot kb<8 -> pos0 chunk kb; slot 8+s -> pos1 chunk 8+s. The slot
        masks the leftmost included sub-block: local i0 = chunk//2, global
        query block G = 4*(i0//2) + h + 2*(i0%2); key block K from perm."""
        p = np.arange(128, dtype=np.float32)
        th = np.empty((128, 16), np.float32)
        pm = perm(h)
        for slot in range(16):
            kb = slot  # slots 0..7: pos0 chunks; slots 8..15: pos1 chunks
            i0 = kb // 2
            G = 4 * (i0 // 2) + h + 2 * (i0 % 2)
            K = 4 * (kb // 4) + pm[kb % 4]
            if G > K:
                th[:, slot] = -1000.0       # fully valid (f16-exact sentinel)
            elif G == K:
                th[:, slot] = p             # causal band diagonal
            else:
                th[:, slot] = 1000.0        # fully invalid
        return th

    def pack_x(xTb, h):
        """[1024, 2048] -> [512, 4096] permuted-block group-major layout."""
        t = xTb.reshape(8, 128, 16, 128)               # (c, p, B, n)
        pm = perm(h)
        blocks = np.array([[4 * g + pm[s] for s in range(4)] for g in range(4)])
        # arr[g, p, c, s, n] = t[c, p, blocks[g, s], n]
        sel = t[:, :, blocks, :]                       # (c, p, g, s, n)
        return np.ascontiguousarray(
            sel.transpose(2, 1, 0, 3, 4).reshape(4 * 128, 8 * 512))

    thrs = [half_thr(0), half_thr(1)]
    in_maps = []
    for core in range(8):
        b, h = core // 2, core % 2
        xTb = np.ascontiguousarray(x[b].T).astype(f16)  # [1024, 2048]
        sm16 = np.concatenate([L.astype(f16), iota, thrs[h].astype(f16)], axis=1)
        in_maps.append({
            "sm16": sm16, "wkd": wkd, "wqv": wqv,
            "xg": pack_x(xTb, h),
        })
    return in_maps


def kernel(x, Wq, Wk, Wv, L_grav):
    import concourse.bass_utils as bass_utils

    if "nc" not in _CACHE:
        _CACHE["nc"] = _build_nc()
    nc = _CACHE["nc"]
    in_maps = _host_inputs(x, Wq, Wk, Wv, L_grav)

    trace = bool(os.environ.get("BASS_KERNEL_TRACE"))
    if trace:
        bass_utils.upload_artifacts = lambda tmpdir: f"file://{tmpdir}"
    res = bass_utils.run_bass_kernel_spmd(nc, in_maps, list(range(8)), trace=trace)
    if trace:
        _CACHE["exec_time_ns"] = res.exec_time_ns
        _CACHE["mean_exec_time_ns"] = res.mean_exec_time_ns

    out = np.empty((B, N, D_HEAD), np.float32)
    for core in range(8):
        b, h = core // 2, core % 2
        r = res.results[core]["out"]
        for i in range(8):  # local block i -> global block G
            G = 4 * (i // 2) + h + 2 * (i % 2)
            out[b, G * 128:(G + 1) * 128] = r[i * 128:(i + 1) * 128]
    return out


# revision 46
# speedup vs baseline: 1.2237x; 1.2237x over previous
"""Bass/Trainium2 kernel for nn_Head_13030930776875 (v3: interleaved).

out = 0.7*softmax(causal(q k^T / sqrt(d))) @ v
    + 0.3*rownorm(causal(exp(-|y_i-y_j|^2 / (2d)))) @ v,   y = k @ L_grav

Sharding: 8 cores = 4 samples x 2 halves. Half h owns query blocks
{h, h+2, ..., h+14} (128-row blocks, stride-2 interleave) — rank-matched
causal needs, so the uniform trim covers both halves with only 72 of the
136 possible 128x128 score tiles per attention (optimal for 2 cores).

Per-core key order is PERMUTED host-side (group g packed as
[4g+h, 4g+1-h, 4g+2+h, 4g+3-h]) so the owned query blocks sit at even
packed positions — queries are strided views of x^T / y^T, no separate
query tensors. Attention over keys is order-invariant; causal masks are
per-core data (one [128,128] mask slot per chunk covers the single
band-or-invalid leftmost sub-block).

Scales are folded host-side (Wq/sqrt(d), L/sqrt(d)) so lang+grav exp
merge into ONE bias-free ACT call per chunk over adjacent PSUM banks;
the grav per-key factor exp(-sq_k/2) rides a g-scaled copy of v_aug.
"""

import math
import os

import numpy as np

B, N, D_MODEL, D_HEAD = 4, 2048, 1024, 128
OMEGA_LANG, OMEGA_GRAV = 0.7, 0.3
NBLK = 16

_CACHE = {}


def _build_nc():
    import concourse.bacc as bacc
    import concourse.mybir as mybir
    import concourse.tile as tile

    dt = mybir.dt
    F16, F32 = dt.float16, dt.float32
    AF = mybir.ActivationFunctionType
    OP = mybir.AluOpType

    nc = bacc.Bacc()

    sm16 = nc.declare_dram_parameter("sm16", [128, 272], F16, isOutput=False)
    wkd = nc.declare_dram_parameter("wkd", [128, 1024], F16, isOutput=False)
    wqv = nc.declare_dram_parameter("wqv", [128, 2 * 1024], F16, isOutput=False)
    # xg[g*128+p, ((c*4+s)*128)+n] = xT[c*128+p, P_g[s]*128+n], permuted blocks
    xg = nc.declare_dram_parameter("xg", [4 * 128, 8 * 512], F16, isOutput=False)
    out_d = nc.declare_dram_parameter("out", [N // 2, 128], F32, isOutput=True)

    with tile.TileContext(nc) as tc:
        with (
            tc.tile_pool(name="big", bufs=1) as big,
            tc.tile_pool(name="xtp", bufs=1) as xtp,
            tc.tile_pool(name="ap", bufs=2) as apool,
            tc.tile_pool(name="small", bufs=4) as small,
            tc.tile_pool(name="outp", bufs=2) as outp,
            tc.tile_pool(name="score", bufs=2, space="PSUM") as score,
            tc.tile_pool(name="pp", bufs=4, space="PSUM") as pp,
        ):
            # PE warmup first: matmuls on a memset tile (no DMA dependency)
            # keep the HAM clock-gate open until real work arrives.
            wtile = big.tile([128, 128], F16, tag="wt")
            nc.vector.memset(wtile[:], 0.5)
            warm = pp.tile([128, 512], F32, tag="pp")
            for i in range(64):
                nc.tensor.matmul(warm[:, 0:128], wtile[:], wtile[:],
                                 start=(i == 0), stop=(i == 63))

            # ---- xg0 first (it gates the first kT proj), then wk/smalls ----
            # x^T permuted-block layout (g, c, j, o, n): o=0 are owned q blocks
            xt_all = xtp.tile([128, 4, 8, 2, 2, 128], F16, tag="xt")

            def load_xg(g):
                nc.sync.dma_start(
                    xt_all[:, g],
                    xg[g * 128:(g + 1) * 128, :].rearrange(
                        "p (c j o n) -> p c j o n", c=8, j=2, o=2))

            load_xg(0)
            wk_s = big.tile([128, 8, 128], F16, tag="wk")
            nc.sync.dma_start(wk_s[:], wkd[:].rearrange("p (c d) -> p c d", c=8))
            sm16_s = big.tile([128, 272], F16, tag="sm16")
            nc.sync.dma_start(sm16_s[:], sm16[:])
            lg_s = sm16_s[:, 0:128]
            io_s = sm16_s[:, 128:256]
            th_s = big.tile([128, 16], F32, tag="th")
            nc.vector.tensor_copy(th_s[:], sm16_s[:, 256:272])
            load_xg(1)
            wp_s = big.tile([128, 2, 8, 128], F16, tag="wp")
            nc.sync.dma_start(wp_s[:], wqv[:].rearrange("p (w c d) -> p w c d",
                                                        w=2, c=8))
            load_xg(2)
            load_xg(3)

            kT = big.tile([128, 16, 128], F16, tag="kT")
            yT = big.tile([128, 8, 2, 128], F16, tag="yT")
            qT = big.tile([128, 1024], F16, tag="qT")
            sqn = big.tile([128, NBLK], F32, tag="sqn")
            g_s = big.tile([128, NBLK], F32, tag="gs")
            vaug = big.tile([128, NBLK, 132], F16, tag="vaug")
            vaug_g = big.tile([128, NBLK, 132], F16, tag="vaugg")

            # causal masks: mk[slot] = (iota128 >= thr[slot]), 16 slots
            mk_s = big.tile([128, 16, 128], F16, tag="mk")
            for slot in range(16):
                nc.vector.tensor_scalar(mk_s[:, slot, :], io_s,
                                        th_s[:, slot:slot + 1], None, OP.is_ge)

            def proj_kt(g):
                ps = pp.tile([128, 512], F32, tag="pp")
                for c in range(8):
                    nc.tensor.matmul(ps[:], wk_s[:, c, :], xt_all[:, g, c],
                                     start=(c == 0), stop=(c == 7))
                nc.vector.tensor_copy(kT[:, 4 * g:4 * (g + 1), :], ps[:])

            def proj_qt(half):
                ps = pp.tile([128, 512], F32, tag="pp")
                for c in range(8):
                    nc.tensor.matmul(ps[:],
                                     wp_s[:, 0, c, :],
                                     xt_all[:, 2 * half:2 * half + 2, c, :, 0, :],
                                     start=(c == 0), stop=(c == 7))
                nc.vector.tensor_copy(qT[:, half * 512:(half + 1) * 512], ps[:])

            def yt_group(g):
                ps = pp.tile([128, 512], F32, tag="pp")
                nc.tensor.matmul(ps[:], lg_s, kT[:, 4 * g:4 * (g + 1), :])
                nc.vector.tensor_copy(yT[:, 2 * g:2 * (g + 1), :, :], ps[:])

            def sqn_chunk(kb):
                ps = pp.tile([128, 512], F32, tag="pp")
                nc.tensor.matmul(ps[:, 0:128], kT[:, kb, :], lg_s)
                scr = small.tile([128, 128], F32, tag="scr")
                nc.scalar.activation(scr[:], ps[:, 0:128], AF.Square,
                                     scale=0.70710678, accum_out=sqn[:, kb:kb + 1])

            def vaug_chunk(kb):
                g, s = kb // 4, kb % 4
                ps = pp.tile([128, 512], F32, tag="pp")
                for c in range(8):
                    nc.tensor.matmul(ps[:, 0:128],
                                     xt_all[:, g, c, s // 2, s % 2, :],
                                     wp_s[:, 1, c, :], start=(c == 0), stop=(c == 7))
                nc.vector.tensor_copy(vaug[:, kb, 0:128], ps[:, 0:128])
                nc.vector.memset(vaug[:, kb, 128:129], 1.0)
                nc.vector.tensor_scalar(vaug_g[:, kb, 0:129], vaug[:, kb, 0:129],
                                        g_s[:, kb:kb + 1], None, OP.mult)

            def chunk(pos, kb, amrg):
                shrink = ((kb // 2) * 128 if pos == 0
                          else max(0, kb // 2 - 4) * 128)
                qsl = qT[:, pos * 512 + shrink:pos * 512 + 512]
                i0 = pos * 4 + shrink // 128
                ysl = yT[:, i0:pos * 4 + 4, 0, :]
                sc = score.tile([128, 1024], F32, tag="sc")
                nc.tensor.matmul(sc[:, shrink:512], kT[:, kb, :], qsl)
                nc.tensor.matmul(sc[:, 512 + shrink:1024],
                                 yT[:, kb // 2, kb % 2, :], ysl)
                nc.scalar.activation(amrg[:, kb, shrink:1024],
                                     sc[:, shrink:1024], AF.Exp)
                # leftmost included sub-block is band/invalid/full per-core
                if pos == 0 or kb >= 8:
                    nc.vector.tensor_tensor(
                        amrg[:, kb, shrink:shrink + 128],
                        amrg[:, kb, shrink:shrink + 128],
                        mk_s[:, kb, :], OP.mult)
                    nc.vector.tensor_tensor(
                        amrg[:, kb, 512 + shrink:512 + shrink + 128],
                        amrg[:, kb, 512 + shrink:512 + shrink + 128],
                        mk_s[:, kb, :], OP.mult)

            def av_block(pos, jj, amrg, obp):
                nkb = 2 * jj + 2 if pos == 0 else 2 * jj + 10
                pol = pp.tile([128, 132], F32, tag="pp")
                pog = pp.tile([128, 132], F32, tag="pp")
                for kb in range(nkb):
                    nc.tensor.matmul(pol[:, 0:129],
                                     amrg[:, kb, jj * 128:(jj + 1) * 128],
                                     vaug[:, kb, 0:129],
                                     start=(kb == 0), stop=(kb == nkb - 1))
                for kb in range(nkb):
                    nc.tensor.matmul(pog[:, 0:129],
                                     amrg[:, kb, 512 + jj * 128:512 + (jj + 1) * 128],
                                     vaug_g[:, kb, 0:129],
                                     start=(kb == 0), stop=(kb == nkb - 1))
                rl = small.tile([128, 1], F32, tag="rl")
                rg = small.tile([128, 1], F32, tag="rg")
                nc.vector.reciprocal(rl[:], pol[:, 128:129])
                nc.vector.tensor_scalar(rl[:], rl[:], OMEGA_LANG, None, OP.mult)
                nc.vector.reciprocal(rg[:], pog[:, 128:129])
                nc.vector.tensor_scalar(rg[:], rg[:], OMEGA_GRAV, None, OP.mult)
                ob = outp.tile([128, 128], F32, tag="ob")
                nc.vector.tensor_scalar(ob[:], pol[:, 0:128], rl[:], None, OP.mult)
                nc.vector.scalar_tensor_tensor(obp[:, jj % 2, :], pog[:, 0:128],
                                               rg[:], ob[:], OP.mult, OP.add)

            def store_pair(pos, pair, obp):
                r0 = pos * 512 + pair * 256
                nc.sync.dma_start(
                    out_d[r0:r0 + 256, :].rearrange("(j p) d -> p j d", j=2),
                    obp[:])

            # ---- attention: pos0 = local query blocks 0..3, pos1 = 4..7 ----
            # AV for block jj is emitted right after its last chunk (nkb-1)
            # so the PE fills EXP gaps and outputs stream out early.
            amrg0 = apool.tile([128, 16, 1024], F16, tag="amrg", name="amrg0")
            amrg1 = apool.tile([128, 16, 1024], F16, tag="amrg", name="amrg1")

            # pos0 prerequisites
            proj_kt(0)
            proj_kt(1)
            proj_qt(0)
            yt_group(0)
            yt_group(1)
            for kb in range(8):
                sqn_chunk(kb)
            nc.scalar.activation(g_s[:, 0:8], sqn[:, 0:8], AF.Exp, scale=-1.0)

            chunk(0, 0, amrg0)
            chunk(0, 1, amrg0)
            # vaug emitted late + spread: fills PE while EXPs drain
            for kb in range(2, 8):
                vaug_chunk(kb - 2)
                chunk(0, kb, amrg0)
            vaug_chunk(6)
            vaug_chunk(7)

            # pos1 prerequisites hoisted before pos0's AVs: they gate pos1's
            # first EXP, while pos0 stores are far from the critical path.
            proj_kt(2)
            proj_kt(3)
            proj_qt(1)
            yt_group(2)
            yt_group(3)
            for kb in range(8, 16):
                sqn_chunk(kb)
            nc.scalar.activation(g_s[:, 8:16], sqn[:, 8:16], AF.Exp, scale=-1.0)

            obp = outp.tile([128, 2, 128], F32, tag="obp")
            for jj in range(4):
                av_block(0, jj, amrg0, obp)
                if jj % 2 == 1:
                    store_pair(0, jj // 2, obp)
                    obp = outp.tile([128, 2, 128], F32, tag="obp")
            chunk(1, 0, amrg1)
            chunk(1, 1, amrg1)
            for kb in range(2, 10):
                vaug_chunk(kb + 6)
                chunk(1, kb, amrg1)
            for kb in range(10, 16):
                chunk(1, kb, amrg1)
            av_block(1, 0, amrg1, obp)
            av_block(1, 1, amrg1, obp)
            store_pair(1, 0, obp)
            obp = outp.tile([128, 2, 128], F32, tag="obp")
            av_block(1, 2, amrg1, obp)
            nc.sync.dma_start(out_d[768:896, :], obp[:, 0, :])
            av_block(1, 3, amrg1, obp)
            nc.sync.dma_start(out_d[896:1024, :], obp[:, 1, :])

    nc.finalize()
    return nc


def _host_inputs(x, Wq, Wk, Wv, L_grav):
    """Build the 8 per-core input maps."""
    f16 = np.float16
    x = np.asarray(x, np.float32)
    s = 1.0 / math.sqrt(D_HEAD)
    Wq = np.asarray(Wq, np.float32) * s        # fold 1/sqrt(d) into Wq
    Wk = np.asarray(Wk, np.float32)
    Wv = np.asarray(Wv, np.float32)
    L = np.asarray(L_grav, np.float32) * s     # fold 1/sqrt(d) into L

    def warr(w):  # [1024,128] -> [128, 8*128] chunk-major for lhsT slices
        return np.ascontiguousarray(
            w.reshape(8, 128, 128).transpose(1, 0, 2).reshape(128, 8 * 128)
        ).astype(f16)

    wkd = warr(Wk)
    wqv = np.concatenate([warr(Wq), warr(Wv)], axis=1)
    iota = np.ascontiguousarray(
        np.broadcast_to(np.arange(128, dtype=np.float32), (128, 128))).astype(f16)

    def perm(h):  # packed block order within each group
        return [h, 1 - h, 2 + h, 3 - h]

    def half_thr(h):
        """thr [128, 16]: mask = iota128 >= thr, one slot per masked chunk.

        Slot kb<8 -> pos0 chunk kb; slot 8+s -> pos1 chunk 8+s. The slot
        masks the leftmost included sub-block: local i0 = chunk//2, global
        query block G = 4*(i0//2) + h + 2*(i0%2); key block K from perm."""
        p = np.arange(128, dtype=np.float32)
        th = np.empty((128, 16), np.float32)
        pm = perm(h)
        for slot in range(16):
            kb = slot  # slots 0..7: pos0 chunks; slots 8..15: pos1 chunks
            i0 = kb // 2
            G = 4 * (i0 // 2) + h + 2 * (i0 % 2)
            K = 4 * (kb // 4) + pm[kb % 4]
            if G > K:
                th[:, slot] = -1000.0       # fully valid (f16-exact sentinel)
            elif G == K:
                th[:, slot] = p             # causal band diagonal
            else:
                th[:, slot] = 1000.0        # fully invalid
        return th

    def pack_x(xTb, h):
        """[1024, 2048] -> [512, 4096] permuted-block group-major layout."""
        t = xTb.reshape(8, 128, 16, 128)               # (c, p, B, n)
        pm = perm(h)
        blocks = np.array([[4 * g + pm[s] for s in range(4)] for g in range(4)])
        # arr[g, p, c, s, n] = t[c, p, blocks[g, s], n]
        sel = t[:, :, blocks, :]                       # (c, p, g, s, n)
        return np.ascontiguousarray(
            sel.transpose(2, 1, 0, 3, 4).reshape(4 * 128, 8 * 512))

    thrs = [half_thr(0), half_thr(1)]
    in_maps = []
    for core in range(8):
        b, h = core // 2, core % 2
        xTb = np.ascontiguousarray(x[b].T).astype(f16)  # [1024, 2048]
        sm16 = np.concatenate([L.astype(f16), iota, thrs[h].astype(f16)], axis=1)
        in_maps.append({
            "sm16": sm16, "wkd": wkd, "wqv": wqv,
            "xg": pack_x(xTb, h),
        })
    return in_maps


def kernel(x, Wq, Wk, Wv, L_grav):
    import concourse.bass_utils as bass_utils

    if "nc" not in _CACHE:
        _CACHE["nc"] = _build_nc()
    nc = _CACHE["nc"]
    in_maps = _host_inputs(x, Wq, Wk, Wv, L_grav)

    trace = bool(os.environ.get("BASS_KERNEL_TRACE"))
    if trace:
        bass_utils.upload_artifacts = lambda tmpdir: f"file://{tmpdir}"
    res = bass_utils.run_bass_kernel_spmd(nc, in_maps, list(range(8)), trace=trace)
    if trace:
        _CACHE["exec_time_ns"] = res.exec_time_ns
        _CACHE["mean_exec_time_ns"] = res.mean_exec_time_ns

    out = np.empty((B, N, D_HEAD), np.float32)
    for core in range(8):
        b, h = core // 2, core % 2
        r = res.results[core]["out"]
        for i in range(8):  # local block i -> global block G
            G = 4 * (i // 2) + h + 2 * (i % 2)
            out[b, G * 128:(G + 1) * 128] = r[i * 128:(i + 1) * 128]
    return out
